# revision 16
# baseline (speedup 1.0000x reference)
"""2-layer GAT on 8 Trainium2 NeuronCores (Bass/Tile).

Strategy (self-contained; shapes hardcoded for N=100000, E=1600000, F=128):
  - Host: add self-loops, sort nodes by in-degree, pack into 128-node blocks
    (degree-homogeneous -> ragged column count K per block, ~1% padding),
    deal blocks round-robin across 8 cores. Per (core, block) build a
    [128 dst-partitions x K columns] gather-index array (int32 rows of a
    node table; padding slots point at a dummy row whose a_src = -60000 so
    its softmax weight underflows to exactly 0).
  - Device, per core: compute fp16 node-table rows [xw1 | a_src1] for its
    own 1/8 of nodes (PE matmuls), AllGather the table to every core's HBM;
    per block: one indirect DMA gathers all incident-edge rows onto the dst
    partitions, then DVE/ACT do leaky-relu/exp/softmax-weighted mean along
    the free axis (dst attention is a per-partition broadcast), ReLU;
    layer-2 table rows [xw2 | a_src2] via PE transpose + matmul; second
    AllGather; repeat edge phase for layer 2 -> out [nloc, 40] fp32.
  - Host: concatenate core outputs, invert the node permutation.
"""

import numpy as np

import concourse.bass as bass
import concourse.tile as tile
from concourse import bacc, mybir
from concourse.bass_utils import run_bass_kernel_spmd
from concourse.masks import make_identity

F16 = mybir.dt.float16
F32 = mybir.dt.float32
I32 = mybir.dt.int32

NEG = 0.2
N_CORES = 8
DUMMY_ASRC = -60000.0


# ----------------------------------------------------------------- host prep
def _prep_layout(edge_index, n_real):
    # self-loops are NOT gathered: their messages use the core-local xw rows
    src_all = np.asarray(edge_index[0], dtype=np.int64)
    dst_all = np.asarray(edge_index[1], dtype=np.int64)

    npad = ((n_real + 128 * N_CORES - 1) // (128 * N_CORES)) * (128 * N_CORES)
    nloc = npad // N_CORES
    nblk = nloc // 128

    deg = np.bincount(dst_all, minlength=npad)
    order = np.argsort(-deg, kind="stable")
    i = np.arange(npad)
    new_id = ((i // 128) % N_CORES) * nloc + (i // (128 * N_CORES)) * 128 + (i % 128)
    new_of_old = np.empty(npad, dtype=np.int64)
    new_of_old[order] = new_id

    ns = new_of_old[src_all]
    nd = new_of_old[dst_all]
    sort_idx = np.argsort(nd, kind="stable")
    snd = nd[sort_idx]
    sns = ns[sort_idx]
    grp_start = np.searchsorted(snd, np.arange(npad))
    j = np.arange(len(snd)) - grp_start[snd]

    cnt = np.bincount(nd, minlength=npad).reshape(N_CORES, nblk, 128)
    K_prog = cnt.max(axis=2).max(axis=0)            # [nblk]
    colofs = np.concatenate([[0], np.cumsum(K_prog)]).astype(np.int64)
    sumk = int(colofs[-1])

    offs = np.full((N_CORES, 128 * sumk), npad, dtype=np.int32)  # dummy = npad
    c_e = snd // nloc
    loc = snd % nloc
    k_e = loc // 128
    p_e = loc % 128
    pos = colofs[k_e] * 128 + p_e * K_prog[k_e] + j
    offs[c_e, pos] = sns.astype(np.int32)
    return dict(npad=npad, nloc=nloc, nblk=nblk, new_of_old=new_of_old,
                K_prog=K_prog.astype(np.int64), colofs=colofs, sumk=sumk,
                offs=offs)


def _fold_schedule(K):
    steps = []
    n = K
    while n > 1:
        half = n // 2
        r = n - half
        steps.append((half, r))   # m[:, :half] += m[:, r:r+half]; n = r
        n = r
    return steps


# ------------------------------------------------------------ program builder
def _build_program(nloc, nblk, npad, K_prog, colofs, sumk):
    nc = bacc.Bacc("TRN2", target_bir_lowering=False, debug=False,
                   enable_asserts=False, num_devices=N_CORES)

    xT_d = nc.dram_tensor("xT", [128, nloc], F16, kind="ExternalInput").ap()
    offs_d = nc.dram_tensor("offs", [128 * sumk], I32, kind="ExternalInput").ap()
    w1_d = nc.dram_tensor("w1", [128, 128], F16, kind="ExternalInput").ap()
    att8_d = nc.dram_tensor("att8", [128, 8], F16, kind="ExternalInput").ap()
    w2e_d = nc.dram_tensor("w2e", [128, 42], F16, kind="ExternalInput").ap()
    b1b_d = nc.dram_tensor("b1b", [128, 128], F16, kind="ExternalInput").ap()
    b2b_d = nc.dram_tensor("b2b", [128, 40], F32, kind="ExternalInput").ap()
    dr1_d = nc.dram_tensor("dr1", [1, 132], F16, kind="ExternalInput").ap()
    dr2_d = nc.dram_tensor("dr2", [1, 41], F16, kind="ExternalInput").ap()
    out_d = nc.dram_tensor("out", [nloc, 40], F32, kind="ExternalOutput").ap()

    t1own = nc.dram_tensor("t1own", [nloc, 132], F16).ap()
    t1full = nc.dram_tensor("t1full", [npad + 1, 132], F16,
                            addr_space="Shared").ap()
    t2own = nc.dram_tensor("t2own", [nloc, 41], F16).ap()
    t2full = nc.dram_tensor("t2full", [npad + 1, 41], F16,
                            addr_space="Shared").ap()

    KMAX = int(max(K_prog))
    rg = [list(range(N_CORES))]

    with tile.TileContext(nc) as tc:
        with (
            tc.tile_pool(name="persist", bufs=1) as pp,
            tc.tile_pool(name="stage", bufs=3) as sp,
            tc.tile_pool(name="gather", bufs=3) as gp,
            tc.tile_pool(name="work", bufs=3) as wp,
            tc.tile_pool(name="small", bufs=4) as smp,
            tc.tile_pool(name="psA", bufs=2, space="PSUM") as psA,
            tc.tile_pool(name="psB", bufs=2, space="PSUM") as psB,
            tc.tile_pool(name="psC", bufs=2, space="PSUM") as psC,
        ):
            # ---- persistent SBUF
            xT = pp.tile([128, nloc], F16)
            xw1T = pp.tile([128, nloc], F16)
            xw1o = pp.tile([128, nblk * 128], F16)
            xw2o = pp.tile([128, nblk * 41], F16)
            offs = pp.tile([128, sumk], I32)
            a_own = pp.tile([128, nblk * 8], F16)   # [.., k*8+ (0:4 src,4:8 dst)]
            ad2 = pp.tile([128, nblk], F16)
            w1 = pp.tile([128, 128], F16)
            att8 = pp.tile([128, 8], F16)
            w2e = pp.tile([128, 42], F16)
            b1b = pp.tile([128, 128], F16)
            b2b = pp.tile([128, 40], F32)
            idm = pp.tile([128, 128], F16)
            drow = pp.tile([1, 132], F16)
            drow2 = pp.tile([1, 41], F16)

            nc.sync.dma_start(xT[:], xT_d)
            nc.sync.dma_start(w1[:], w1_d)
            nc.sync.dma_start(att8[:], att8_d)
            nc.sync.dma_start(w2e[:], w2e_d)
            nc.sync.dma_start(b1b[:], b1b_d)
            nc.sync.dma_start(b2b[:], b2b_d)
            nc.sync.dma_start(drow[:], dr1_d)
            nc.sync.dma_start(drow2[:], dr2_d)
            make_identity(nc, idm[:])
            for k in range(nblk):
                c0, K = int(colofs[k]), int(K_prog[k])
                nc.sync.dma_start(
                    offs[:, c0:c0 + K],
                    offs_d[c0 * 128:(c0 + K) * 128].rearrange(
                        "(p j) -> p j", p=128))

            # ---- phase A: xw1T (hc-major), then per block xw1 rows + attn
            for s in range(0, nloc, 512):
                wdt = min(512, nloc - s)
                ps = psB.tile([128, wdt], F32, tag="psxw1T")
                nc.tensor.matmul(ps[:], lhsT=w1[:], rhs=xT[:, s:s + wdt],
                                 start=True, stop=True)
                nc.vector.tensor_copy(xw1T[:, s:s + wdt], ps[:])
            for k in range(nblk):
                ps_xw = psA.tile([128, 128], F32, tag="psbig")
                nc.tensor.matmul(ps_xw[:], lhsT=xT[:, k * 128:(k + 1) * 128],
                                 rhs=w1[:], start=True, stop=True)
                ps_a = psC.tile([128, 8], F32, tag="pssmall")
                nc.tensor.matmul(ps_a[:], lhsT=xw1T[:, k * 128:(k + 1) * 128],
                                 rhs=att8[:], start=True, stop=True)
                stg = sp.tile([128, 132], F16, tag="stg1")
                nc.scalar.copy(stg[:, 0:128], ps_xw[:])
                nc.vector.tensor_copy(xw1o[:, k * 128:(k + 1) * 128], ps_xw[:])
                nc.vector.tensor_copy(stg[:, 128:132], ps_a[:, 0:4])
                nc.vector.tensor_copy(a_own[:, k * 8:k * 8 + 8], ps_a[:])
                nc.sync.dma_start(t1own[k * 128:(k + 1) * 128, :], stg[:])

            # ---- AllGather layer-1 table + dummy row
            nc.gpsimd.collective_compute(
                "AllGather", mybir.AluOpType.bypass, replica_groups=rg,
                ins=[t1own.opt()], outs=[t1full[0:npad, :].opt()])
            nc.sync.dma_start(t1full[npad:npad + 1, :], drow[:])

            # ---- phase B/C: layer-1 edges + layer-2 node table, per block
            for k in range(nblk):
                c0, K = int(colofs[k]), int(K_prog[k])
                ev = wp.tile([128, (K + 1) * 4], F16, tag="e1")
                e3 = ev[:].rearrange("p (k h) -> p k h", h=4)
                if K > 0:
                    g1 = gp.tile([128, K * 132], F16, tag="g1")
                    for j in range(K):
                        nc.gpsimd.indirect_dma_start(
                            out=g1[:, j * 132:(j + 1) * 132], out_offset=None,
                            in_=t1full[:, :],
                            in_offset=bass.IndirectOffsetOnAxis(
                                ap=offs[:, c0 + j:c0 + j + 1], axis=0))
                    g3 = g1[:].rearrange("p (k f) -> p k f", f=132)
                    nc.vector.tensor_tensor(
                        out=e3[:, 0:K, :], in0=g3[:, :, 128:132],
                        in1=a_own[:, k * 8 + 4:k * 8 + 8][:, None, :]
                            .broadcast_to([128, K, 4]),
                        op=mybir.AluOpType.add)
                nc.vector.tensor_tensor(
                    out=ev[:, K * 4:(K + 1) * 4],
                    in0=a_own[:, k * 8:k * 8 + 4],
                    in1=a_own[:, k * 8 + 4:k * 8 + 8],
                    op=mybir.AluOpType.add)
                nc.vector.scalar_tensor_tensor(
                    out=e3, in0=e3, scalar=NEG, in1=e3,
                    op0=mybir.AluOpType.mult, op1=mybir.AluOpType.max)
                wv = wp.tile([128, (K + 1) * 4], F16, tag="w1v")
                nc.scalar.activation(out=wv[:], in_=ev[:],
                                     func=mybir.ActivationFunctionType.Exp)
                w3 = wv[:].rearrange("p (k h) -> p k h", h=4)
                mv = wp.tile([128, (K + 1) * 128], F16, tag="m1")
                m4 = mv[:].rearrange("p (k h c) -> p k h c", h=4, c=32)
                if K > 0:
                    nc.vector.tensor_tensor(
                        out=m4[:, 0:K],
                        in0=g3[:, :, 0:128].rearrange(
                            "p k (h c) -> p k h c", h=4),
                        in1=w3[:, 0:K, :, None].broadcast_to([128, K, 4, 32]),
                        op=mybir.AluOpType.mult)
                nc.vector.tensor_tensor(
                    out=mv[:, K * 128:(K + 1) * 128]
                        .rearrange("p (h c) -> p h c", h=4),
                    in0=xw1o[:, k * 128:(k + 1) * 128]
                        .rearrange("p (h c) -> p h c", h=4),
                    in1=wv[:, K * 4:(K + 1) * 4][:, :, None]
                        .broadcast_to([128, 4, 32]),
                    op=mybir.AluOpType.mult)
                for half, r in _fold_schedule(K + 1):
                    nc.vector.tensor_tensor(
                        out=mv[:, 0:half * 128], in0=mv[:, 0:half * 128],
                        in1=mv[:, r * 128:(r + half) * 128],
                        op=mybir.AluOpType.add)
                den = smp.tile([128, 4], F32, tag="den1")
                nc.vector.tensor_reduce(
                    out=den[:], in_=wv[:].rearrange("p (k h) -> p h k", h=4),
                    axis=mybir.AxisListType.X, op=mybir.AluOpType.add)
                rden = smp.tile([128, 4], F16, tag="rden1")
                with nc.allow_low_precision(reason="softmax denom recip f16"):
                    nc.vector.reciprocal(rden[:], den[:])
                hb = sp.tile([128, 128], F16, tag="hb")
                nc.vector.tensor_tensor(
                    out=hb[:].rearrange("p (h c) -> p h c", h=4),
                    in0=mv[:, 0:128].rearrange("p (h c) -> p h c", h=4),
                    in1=rden[:][:, :, None].broadcast_to([128, 4, 32]),
                    op=mybir.AluOpType.mult)
                nc.vector.tensor_tensor(out=hb[:], in0=hb[:], in1=b1b[:],
                                        op=mybir.AluOpType.add)
                hcl = sp.tile([128, 128], F16, tag="hcl")
                nc.scalar.activation(out=hcl[:], in_=hb[:],
                                     func=mybir.ActivationFunctionType.Relu)
                # layer-2 node rows for this block
                ps_t = psA.tile([128, 128], F16, tag="psbig")
                nc.tensor.transpose(out=ps_t[:], in_=hcl[:], identity=idm[:])
                hT = sp.tile([128, 128], F16, tag="hT")
                nc.vector.tensor_copy(hT[:], ps_t[:])
                ps2 = psC.tile([128, 42], F32, tag="pssmall")
                nc.tensor.matmul(ps2[:], lhsT=hT[:], rhs=w2e[:],
                                 start=True, stop=True)
                stg2 = sp.tile([128, 41], F16, tag="stg2")
                nc.scalar.copy(stg2[:], ps2[:, 0:41])
                nc.vector.tensor_copy(xw2o[:, k * 41:(k + 1) * 41],
                                      ps2[:, 0:41])
                nc.vector.tensor_copy(ad2[:, k:k + 1], ps2[:, 41:42])
                nc.sync.dma_start(t2own[k * 128:(k + 1) * 128, :], stg2[:])

            # ---- AllGather layer-2 table + dummy row
            nc.gpsimd.collective_compute(
                "AllGather", mybir.AluOpType.bypass, replica_groups=rg,
                ins=[t2own.opt()], outs=[t2full[0:npad, :].opt()])
            nc.sync.dma_start(t2full[npad:npad + 1, :], drow2[:])

            # ---- phase D: layer-2 edges
            for k in range(nblk):
                c0, K = int(colofs[k]), int(K_prog[k])
                e2 = wp.tile([128, K + 1], F16, tag="e2")
                if K > 0:
                    g2 = gp.tile([128, K * 41], F16, tag="g2")
                    for j in range(K):
                        nc.gpsimd.indirect_dma_start(
                            out=g2[:, j * 41:(j + 1) * 41], out_offset=None,
                            in_=t2full[:, :],
                            in_offset=bass.IndirectOffsetOnAxis(
                                ap=offs[:, c0 + j:c0 + j + 1], axis=0))
                    g23 = g2[:].rearrange("p (k f) -> p k f", f=41)
                    nc.vector.tensor_tensor(
                        out=e2[:, 0:K][:, :, None], in0=g23[:, :, 40:41],
                        in1=ad2[:, k:k + 1][:, None, :]
                            .broadcast_to([128, K, 1]),
                        op=mybir.AluOpType.add)
                nc.vector.tensor_tensor(
                    out=e2[:, K:K + 1],
                    in0=xw2o[:, k * 41 + 40:k * 41 + 41],
                    in1=ad2[:, k:k + 1],
                    op=mybir.AluOpType.add)
                nc.vector.scalar_tensor_tensor(
                    out=e2[:], in0=e2[:], scalar=NEG, in1=e2[:],
                    op0=mybir.AluOpType.mult, op1=mybir.AluOpType.max)
                w2v = wp.tile([128, K + 1], F16, tag="w2v")
                nc.scalar.activation(out=w2v[:], in_=e2[:],
                                     func=mybir.ActivationFunctionType.Exp)
                m2 = wp.tile([128, (K + 1) * 40], F16, tag="m2")
                if K > 0:
                    nc.vector.tensor_tensor(
                        out=m2[:, 0:K * 40].rearrange(
                            "p (k c) -> p k c", c=40),
                        in0=g23[:, :, 0:40],
                        in1=w2v[:, 0:K][:, :, None].broadcast_to([128, K, 40]),
                        op=mybir.AluOpType.mult)
                nc.vector.tensor_tensor(
                    out=m2[:, K * 40:(K + 1) * 40],
                    in0=xw2o[:, k * 41:k * 41 + 40],
                    in1=w2v[:, K:K + 1].broadcast_to([128, 40]),
                    op=mybir.AluOpType.mult)
                for half, r in _fold_schedule(K + 1):
                    nc.vector.tensor_tensor(
                        out=m2[:, 0:half * 40], in0=m2[:, 0:half * 40],
                        in1=m2[:, r * 40:(r + half) * 40],
                        op=mybir.AluOpType.add)
                den2 = smp.tile([128, 1], F32, tag="den2")
                nc.vector.tensor_reduce(out=den2[:], in_=w2v[:],
                                        axis=mybir.AxisListType.X,
                                        op=mybir.AluOpType.add)
                rden2 = smp.tile([128, 1], F16, tag="rden2")
                with nc.allow_low_precision(reason="softmax denom recip f16"):
                    nc.vector.reciprocal(rden2[:], den2[:])
                o1 = sp.tile([128, 40], F32, tag="o1")
                nc.vector.tensor_tensor(
                    out=o1[:], in0=m2[:, 0:40],
                    in1=rden2[:, 0:1].broadcast_to([128, 40]),
                    op=mybir.AluOpType.mult)
                nc.vector.tensor_tensor(out=o1[:], in0=o1[:], in1=b2b[:],
                                        op=mybir.AluOpType.add)
                nc.sync.dma_start(out_d[k * 128:(k + 1) * 128, :], o1[:])

    nc.compile()
    return nc


_CACHE = {}


def _get_program(nloc, nblk, npad, K_prog, colofs, sumk):
    key = (nloc, npad, tuple(int(x) for x in K_prog))
    if key not in _CACHE:
        _CACHE[key] = _build_program(nloc, nblk, npad, K_prog, colofs, sumk)
    return _CACHE[key]


# ------------------------------------------------------------------ kernel
def kernel(x, edge_index, W1, att_src1, att_dst1, b1, W2, att_src2, att_dst2,
           b2):
    x = np.asarray(x, dtype=np.float32)
    n_real = x.shape[0]
    lay = _prep_layout(np.asarray(edge_index), n_real)
    npad, nloc, nblk = lay["npad"], lay["nloc"], lay["nblk"]
    nof = lay["new_of_old"]

    xp = np.zeros((npad, x.shape[1]), dtype=np.float16)
    xp[nof[:n_real]] = x.astype(np.float16)

    W1 = np.asarray(W1, np.float32)
    W2 = np.asarray(W2, np.float32)
    att_src1 = np.asarray(att_src1, np.float32)
    att_dst1 = np.asarray(att_dst1, np.float32)
    att_src2 = np.asarray(att_src2, np.float32)
    att_dst2 = np.asarray(att_dst2, np.float32)
    b1 = np.asarray(b1, np.float32)
    b2 = np.asarray(b2, np.float32)

    att8 = np.zeros((128, 8), dtype=np.float16)
    for h in range(4):
        att8[h * 32:(h + 1) * 32, h] = att_src1[h].astype(np.float16)
        att8[h * 32:(h + 1) * 32, 4 + h] = att_dst1[h].astype(np.float16)
    w2e = np.concatenate(
        [W2, (W2 @ att_src2[0])[:, None], (W2 @ att_dst2[0])[:, None]],
        axis=1).astype(np.float16)                       # [128, 42]
    b1b = np.broadcast_to(b1.astype(np.float16), (128, 128)).copy()
    b2b = np.broadcast_to(b2.astype(np.float32), (128, 40)).copy()
    dr1 = np.zeros((1, 132), np.float16)
    dr1[0, 128:132] = DUMMY_ASRC
    dr2 = np.zeros((1, 41), np.float16)
    dr2[0, 40] = DUMMY_ASRC

    nc = _get_program(nloc, nblk, npad, lay["K_prog"], lay["colofs"],
                      lay["sumk"])

    in_maps = []
    for c in range(N_CORES):
        xT_c = np.ascontiguousarray(
            xp[c * nloc:(c + 1) * nloc].T)               # [128, nloc] f16
        in_maps.append({
            "xT": xT_c,
            "offs": lay["offs"][c],
            "w1": W1.astype(np.float16),
            "att8": att8,
            "w2e": w2e,
            "b1b": b1b,
            "b2b": b2b,
            "dr1": dr1,
            "dr2": dr2,
        })

    res = run_bass_kernel_spmd(nc, in_maps, core_ids=list(range(N_CORES)))
    full = np.concatenate([res.results[c]["out"] for c in range(N_CORES)],
                          axis=0)                        # [npad, 40]
    return np.ascontiguousarray(full[nof[:n_real]]).astype(np.float32)


# revision 17
# speedup vs baseline: 1.1929x; 1.1929x over previous
"""2-layer GAT on 8 Trainium2 NeuronCores (Bass/Tile).

Strategy (self-contained; shapes hardcoded for N=100000, E=1600000, F=128):
  - Host: add self-loops, sort nodes by in-degree, pack into 128-node blocks
    (degree-homogeneous -> ragged column count K per block, ~1% padding),
    deal blocks round-robin across 8 cores. Per (core, block) build a
    [128 dst-partitions x K columns] gather-index array (int32 rows of a
    node table; padding slots point at a dummy row whose a_src = -60000 so
    its softmax weight underflows to exactly 0).
  - Device, per core: compute fp16 node-table rows [xw1 | a_src1] for its
    own 1/8 of nodes (PE matmuls), AllGather the table to every core's HBM;
    per block: one indirect DMA gathers all incident-edge rows onto the dst
    partitions, then DVE/ACT do leaky-relu/exp/softmax-weighted mean along
    the free axis (dst attention is a per-partition broadcast), ReLU;
    layer-2 table rows [xw2 | a_src2] via PE transpose + matmul; second
    AllGather; repeat edge phase for layer 2 -> out [nloc, 40] fp32.
  - Host: concatenate core outputs, invert the node permutation.
"""

import numpy as np

import concourse.bass as bass
import concourse.tile as tile
from concourse import bacc, mybir
from concourse.bass_utils import run_bass_kernel_spmd
from concourse.masks import make_identity

F16 = mybir.dt.float16
F32 = mybir.dt.float32
I32 = mybir.dt.int32

NEG = 0.2
N_CORES = 8
DUMMY_ASRC = -60000.0


# ----------------------------------------------------------------- host prep
def _prep_layout(edge_index, n_real):
    # self-loops are NOT gathered: their messages use the core-local xw rows
    src_all = np.asarray(edge_index[0], dtype=np.int64)
    dst_all = np.asarray(edge_index[1], dtype=np.int64)

    npad = ((n_real + 128 * N_CORES - 1) // (128 * N_CORES)) * (128 * N_CORES)
    nloc = npad // N_CORES
    nblk = nloc // 128

    deg = np.bincount(dst_all, minlength=npad)
    order = np.argsort(-deg, kind="stable")
    i = np.arange(npad)
    new_id = ((i // 128) % N_CORES) * nloc + (i // (128 * N_CORES)) * 128 + (i % 128)
    new_of_old = np.empty(npad, dtype=np.int64)
    new_of_old[order] = new_id

    ns = new_of_old[src_all]
    nd = new_of_old[dst_all]
    sort_idx = np.argsort(nd, kind="stable")
    snd = nd[sort_idx]
    sns = ns[sort_idx]
    grp_start = np.searchsorted(snd, np.arange(npad))
    j = np.arange(len(snd)) - grp_start[snd]

    cnt = np.bincount(nd, minlength=npad).reshape(N_CORES, nblk, 128)
    K_prog = cnt.max(axis=2).max(axis=0)            # [nblk]
    colofs = np.concatenate([[0], np.cumsum(K_prog)]).astype(np.int64)
    sumk = int(colofs[-1])

    offs = np.full((N_CORES, 128 * sumk), npad, dtype=np.int32)  # dummy = npad
    c_e = snd // nloc
    loc = snd % nloc
    k_e = loc // 128
    p_e = loc % 128
    pos = p_e * sumk + colofs[k_e] + j
    offs[c_e, pos] = sns.astype(np.int32)
    return dict(npad=npad, nloc=nloc, nblk=nblk, new_of_old=new_of_old,
                K_prog=K_prog.astype(np.int64), colofs=colofs, sumk=sumk,
                offs=offs)


def _fold_schedule(K):
    steps = []
    n = K
    while n > 1:
        half = n // 2
        r = n - half
        steps.append((half, r))   # m[:, :half] += m[:, r:r+half]; n = r
        n = r
    return steps


# ------------------------------------------------------------ program builder
def _build_program(nloc, nblk, npad, K_prog, colofs, sumk):
    nc = bacc.Bacc("TRN2", target_bir_lowering=False, debug=False,
                   enable_asserts=False, num_devices=N_CORES)

    xT_d = nc.dram_tensor("xT", [128, nloc], F16, kind="ExternalInput").ap()
    offs_d = nc.dram_tensor("offs", [128 * sumk], I32, kind="ExternalInput").ap()
    w1_d = nc.dram_tensor("w1", [128, 128], F16, kind="ExternalInput").ap()
    att8_d = nc.dram_tensor("att8", [128, 8], F16, kind="ExternalInput").ap()
    w2e_d = nc.dram_tensor("w2e", [128, 42], F16, kind="ExternalInput").ap()
    b1b_d = nc.dram_tensor("b1b", [128, 128], F16, kind="ExternalInput").ap()
    b2b_d = nc.dram_tensor("b2b", [128, 40], F32, kind="ExternalInput").ap()
    dr1_d = nc.dram_tensor("dr1", [1, 132], F16, kind="ExternalInput").ap()
    dr2_d = nc.dram_tensor("dr2", [1, 41], F16, kind="ExternalInput").ap()
    out_d = nc.dram_tensor("out", [nloc, 40], F32, kind="ExternalOutput").ap()

    t1own = nc.dram_tensor("t1own", [nloc, 132], F16).ap()
    t1full = nc.dram_tensor("t1full", [npad + 1, 132], F16,
                            addr_space="Shared").ap()
    t2own = nc.dram_tensor("t2own", [nloc, 41], F16).ap()
    t2full = nc.dram_tensor("t2full", [npad + 1, 41], F16,
                            addr_space="Shared").ap()

    KMAX = int(max(K_prog))
    rg = [list(range(N_CORES))]

    with tile.TileContext(nc) as tc:
        with (
            tc.tile_pool(name="persist", bufs=1) as pp,
            tc.tile_pool(name="stage", bufs=3) as sp,
            tc.tile_pool(name="gather", bufs=3) as gp,
            tc.tile_pool(name="work", bufs=3) as wp,
            tc.tile_pool(name="small", bufs=4) as smp,
            tc.tile_pool(name="psA", bufs=2, space="PSUM") as psA,
            tc.tile_pool(name="psB", bufs=2, space="PSUM") as psB,
            tc.tile_pool(name="psC", bufs=2, space="PSUM") as psC,
        ):
            # ---- persistent SBUF
            xT = pp.tile([128, nloc], F16)
            xw1T = pp.tile([128, nloc], F16)
            xw1o = pp.tile([128, nblk * 128], F16)
            xw2o = pp.tile([128, nblk * 41], F16)
            offs = pp.tile([128, sumk], I32)
            a_own = pp.tile([128, nblk * 8], F16)   # [.., k*8+ (0:4 src,4:8 dst)]
            ad2 = pp.tile([128, nblk], F16)
            w1 = pp.tile([128, 128], F16)
            att8 = pp.tile([128, 8], F16)
            w2e = pp.tile([128, 42], F16)
            b1b = pp.tile([128, 128], F16)
            b2b = pp.tile([128, 40], F32)
            idm = pp.tile([128, 128], F16)
            drow = pp.tile([1, 132], F16)
            drow2 = pp.tile([1, 41], F16)

            nc.sync.dma_start(xT[:], xT_d)
            nc.sync.dma_start(w1[:], w1_d)
            nc.sync.dma_start(att8[:], att8_d)
            nc.sync.dma_start(w2e[:], w2e_d)
            nc.sync.dma_start(b1b[:], b1b_d)
            nc.sync.dma_start(b2b[:], b2b_d)
            nc.sync.dma_start(drow[:], dr1_d)
            nc.sync.dma_start(drow2[:], dr2_d)
            make_identity(nc, idm[:])
            nc.sync.dma_start(
                offs[:], offs_d.rearrange("(p j) -> p j", p=128))

            # ---- phase A: xw1T (hc-major), then per block xw1 rows + attn
            for s in range(0, nloc, 512):
                wdt = min(512, nloc - s)
                ps = psB.tile([128, wdt], F32, tag="psxw1T")
                nc.tensor.matmul(ps[:], lhsT=w1[:], rhs=xT[:, s:s + wdt],
                                 start=True, stop=True)
                nc.vector.tensor_copy(xw1T[:, s:s + wdt], ps[:])
            for k in range(nblk):
                ps_xw = psA.tile([128, 128], F32, tag="psbig")
                nc.tensor.matmul(ps_xw[:], lhsT=xT[:, k * 128:(k + 1) * 128],
                                 rhs=w1[:], start=True, stop=True)
                ps_a = psC.tile([128, 8], F32, tag="pssmall")
                nc.tensor.matmul(ps_a[:], lhsT=xw1T[:, k * 128:(k + 1) * 128],
                                 rhs=att8[:], start=True, stop=True)
                stg = sp.tile([128, 132], F16, tag="stg1")
                nc.scalar.copy(stg[:, 0:128], ps_xw[:])
                nc.vector.tensor_copy(xw1o[:, k * 128:(k + 1) * 128],
                                      stg[:, 0:128])
                nc.vector.tensor_copy(stg[:, 128:132], ps_a[:, 0:4])
                nc.vector.tensor_copy(a_own[:, k * 8:k * 8 + 8], ps_a[:])
                nc.sync.dma_start(t1own[k * 128:(k + 1) * 128, :], stg[:])

            # ---- AllGather layer-1 table + dummy row
            nc.gpsimd.collective_compute(
                "AllGather", mybir.AluOpType.bypass, replica_groups=rg,
                ins=[t1own.opt()], outs=[t1full[0:npad, :].opt()])
            nc.sync.dma_start(t1full[npad:npad + 1, :], drow[:])

            # ---- phase B/C: layer-1 edges + layer-2 node table, per block
            for k in range(nblk):
                c0, K = int(colofs[k]), int(K_prog[k])
                ev = wp.tile([128, (K + 1) * 4], F16, tag="e1")
                e3 = ev[:].rearrange("p (k h) -> p k h", h=4)
                if K > 0:
                    g1 = gp.tile([128, K * 132], F16, tag="g1")
                    for j in range(K):
                        nc.gpsimd.indirect_dma_start(
                            out=g1[:, j * 132:(j + 1) * 132], out_offset=None,
                            in_=t1full[:, :],
                            in_offset=bass.IndirectOffsetOnAxis(
                                ap=offs[:, c0 + j:c0 + j + 1], axis=0))
                    g3 = g1[:].rearrange("p (k f) -> p k f", f=132)
                    nc.vector.tensor_tensor(
                        out=e3[:, 0:K, :], in0=g3[:, :, 128:132],
                        in1=a_own[:, k * 8 + 4:k * 8 + 8][:, None, :]
                            .broadcast_to([128, K, 4]),
                        op=mybir.AluOpType.add)
                nc.vector.tensor_tensor(
                    out=ev[:, K * 4:(K + 1) * 4],
                    in0=a_own[:, k * 8:k * 8 + 4],
                    in1=a_own[:, k * 8 + 4:k * 8 + 8],
                    op=mybir.AluOpType.add)
                nc.vector.scalar_tensor_tensor(
                    out=e3, in0=e3, scalar=NEG, in1=e3,
                    op0=mybir.AluOpType.mult, op1=mybir.AluOpType.max)
                wv = wp.tile([128, (K + 1) * 4], F16, tag="w1v")
                nc.scalar.activation(out=wv[:], in_=ev[:],
                                     func=mybir.ActivationFunctionType.Exp)
                w3 = wv[:].rearrange("p (k h) -> p k h", h=4)
                mv = wp.tile([128, (K + 1) * 128], F16, tag="m1")
                m4 = mv[:].rearrange("p (k h c) -> p k h c", h=4, c=32)
                if K > 0:
                    nc.vector.tensor_tensor(
                        out=m4[:, 0:K],
                        in0=g3[:, :, 0:128].rearrange(
                            "p k (h c) -> p k h c", h=4),
                        in1=w3[:, 0:K, :, None].broadcast_to([128, K, 4, 32]),
                        op=mybir.AluOpType.mult)
                nc.vector.tensor_tensor(
                    out=mv[:, K * 128:(K + 1) * 128]
                        .rearrange("p (h c) -> p h c", h=4),
                    in0=xw1o[:, k * 128:(k + 1) * 128]
                        .rearrange("p (h c) -> p h c", h=4),
                    in1=wv[:, K * 4:(K + 1) * 4][:, :, None]
                        .broadcast_to([128, 4, 32]),
                    op=mybir.AluOpType.mult)
                for half, r in _fold_schedule(K + 1):
                    nc.vector.tensor_tensor(
                        out=mv[:, 0:half * 128], in0=mv[:, 0:half * 128],
                        in1=mv[:, r * 128:(r + half) * 128],
                        op=mybir.AluOpType.add)
                den = smp.tile([128, 4], F32, tag="den1")
                nc.vector.tensor_reduce(
                    out=den[:], in_=wv[:].rearrange("p (k h) -> p h k", h=4),
                    axis=mybir.AxisListType.X, op=mybir.AluOpType.add)
                rden = smp.tile([128, 4], F16, tag="rden1")
                with nc.allow_low_precision(reason="softmax denom recip f16"):
                    nc.vector.reciprocal(rden[:], den[:])
                hb = sp.tile([128, 128], F16, tag="hb")
                nc.vector.tensor_tensor(
                    out=hb[:].rearrange("p (h c) -> p h c", h=4),
                    in0=mv[:, 0:128].rearrange("p (h c) -> p h c", h=4),
                    in1=rden[:][:, :, None].broadcast_to([128, 4, 32]),
                    op=mybir.AluOpType.mult)
                nc.vector.tensor_tensor(out=hb[:], in0=hb[:], in1=b1b[:],
                                        op=mybir.AluOpType.add)
                hcl = sp.tile([128, 128], F16, tag="hcl")
                nc.scalar.activation(out=hcl[:], in_=hb[:],
                                     func=mybir.ActivationFunctionType.Relu)
                # layer-2 node rows for this block
                ps_t = psA.tile([128, 128], F16, tag="psbig")
                nc.tensor.transpose(out=ps_t[:], in_=hcl[:], identity=idm[:])
                hT = sp.tile([128, 128], F16, tag="hT")
                nc.vector.tensor_copy(hT[:], ps_t[:])
                ps2 = psC.tile([128, 42], F32, tag="pssmall")
                nc.tensor.matmul(ps2[:], lhsT=hT[:], rhs=w2e[:],
                                 start=True, stop=True)
                stg2 = sp.tile([128, 41], F16, tag="stg2")
                nc.scalar.copy(stg2[:], ps2[:, 0:41])
                nc.vector.tensor_copy(xw2o[:, k * 41:(k + 1) * 41], stg2[:])
                nc.vector.tensor_copy(ad2[:, k:k + 1], ps2[:, 41:42])
                nc.sync.dma_start(t2own[k * 128:(k + 1) * 128, :], stg2[:])

            # ---- AllGather layer-2 table + dummy row
            nc.gpsimd.collective_compute(
                "AllGather", mybir.AluOpType.bypass, replica_groups=rg,
                ins=[t2own.opt()], outs=[t2full[0:npad, :].opt()])
            nc.sync.dma_start(t2full[npad:npad + 1, :], drow2[:])

            # ---- phase D: layer-2 edges
            for k in range(nblk):
                c0, K = int(colofs[k]), int(K_prog[k])
                e2 = wp.tile([128, K + 1], F16, tag="e2")
                if K > 0:
                    g2 = gp.tile([128, K * 41], F16, tag="g2")
                    for j in range(K):
                        nc.gpsimd.indirect_dma_start(
                            out=g2[:, j * 41:(j + 1) * 41], out_offset=None,
                            in_=t2full[:, :],
                            in_offset=bass.IndirectOffsetOnAxis(
                                ap=offs[:, c0 + j:c0 + j + 1], axis=0))
                    g23 = g2[:].rearrange("p (k f) -> p k f", f=41)
                    nc.vector.tensor_tensor(
                        out=e2[:, 0:K][:, :, None], in0=g23[:, :, 40:41],
                        in1=ad2[:, k:k + 1][:, None, :]
                            .broadcast_to([128, K, 1]),
                        op=mybir.AluOpType.add)
                nc.vector.tensor_tensor(
                    out=e2[:, K:K + 1],
                    in0=xw2o[:, k * 41 + 40:k * 41 + 41],
                    in1=ad2[:, k:k + 1],
                    op=mybir.AluOpType.add)
                nc.vector.scalar_tensor_tensor(
                    out=e2[:], in0=e2[:], scalar=NEG, in1=e2[:],
                    op0=mybir.AluOpType.mult, op1=mybir.AluOpType.max)
                w2v = wp.tile([128, K + 1], F16, tag="w2v")
                nc.scalar.activation(out=w2v[:], in_=e2[:],
                                     func=mybir.ActivationFunctionType.Exp)
                m2 = wp.tile([128, (K + 1) * 40], F16, tag="m2")
                if K > 0:
                    nc.vector.tensor_tensor(
                        out=m2[:, 0:K * 40].rearrange(
                            "p (k c) -> p k c", c=40),
                        in0=g23[:, :, 0:40],
                        in1=w2v[:, 0:K][:, :, None].broadcast_to([128, K, 40]),
                        op=mybir.AluOpType.mult)
                nc.vector.tensor_tensor(
                    out=m2[:, K * 40:(K + 1) * 40],
                    in0=xw2o[:, k * 41:k * 41 + 40],
                    in1=w2v[:, K:K + 1].broadcast_to([128, 40]),
                    op=mybir.AluOpType.mult)
                for half, r in _fold_schedule(K + 1):
                    nc.vector.tensor_tensor(
                        out=m2[:, 0:half * 40], in0=m2[:, 0:half * 40],
                        in1=m2[:, r * 40:(r + half) * 40],
                        op=mybir.AluOpType.add)
                den2 = smp.tile([128, 1], F32, tag="den2")
                nc.vector.tensor_reduce(out=den2[:], in_=w2v[:],
                                        axis=mybir.AxisListType.X,
                                        op=mybir.AluOpType.add)
                rden2 = smp.tile([128, 1], F16, tag="rden2")
                with nc.allow_low_precision(reason="softmax denom recip f16"):
                    nc.vector.reciprocal(rden2[:], den2[:])
                o1 = sp.tile([128, 40], F32, tag="o1")
                nc.vector.tensor_tensor(
                    out=o1[:], in0=m2[:, 0:40],
                    in1=rden2[:, 0:1].broadcast_to([128, 40]),
                    op=mybir.AluOpType.mult)
                nc.vector.tensor_tensor(out=o1[:], in0=o1[:], in1=b2b[:],
                                        op=mybir.AluOpType.add)
                nc.sync.dma_start(out_d[k * 128:(k + 1) * 128, :], o1[:])

    nc.compile()
    return nc


_CACHE = {}


def _get_program(nloc, nblk, npad, K_prog, colofs, sumk):
    key = (nloc, npad, tuple(int(x) for x in K_prog))
    if key not in _CACHE:
        _CACHE[key] = _build_program(nloc, nblk, npad, K_prog, colofs, sumk)
    return _CACHE[key]


# ------------------------------------------------------------------ kernel
def kernel(x, edge_index, W1, att_src1, att_dst1, b1, W2, att_src2, att_dst2,
           b2):
    x = np.asarray(x, dtype=np.float32)
    n_real = x.shape[0]
    lay = _prep_layout(np.asarray(edge_index), n_real)
    npad, nloc, nblk = lay["npad"], lay["nloc"], lay["nblk"]
    nof = lay["new_of_old"]

    xp = np.zeros((npad, x.shape[1]), dtype=np.float16)
    xp[nof[:n_real]] = x.astype(np.float16)

    W1 = np.asarray(W1, np.float32)
    W2 = np.asarray(W2, np.float32)
    att_src1 = np.asarray(att_src1, np.float32)
    att_dst1 = np.asarray(att_dst1, np.float32)
    att_src2 = np.asarray(att_src2, np.float32)
    att_dst2 = np.asarray(att_dst2, np.float32)
    b1 = np.asarray(b1, np.float32)
    b2 = np.asarray(b2, np.float32)

    att8 = np.zeros((128, 8), dtype=np.float16)
    for h in range(4):
        att8[h * 32:(h + 1) * 32, h] = att_src1[h].astype(np.float16)
        att8[h * 32:(h + 1) * 32, 4 + h] = att_dst1[h].astype(np.float16)
    w2e = np.concatenate(
        [W2, (W2 @ att_src2[0])[:, None], (W2 @ att_dst2[0])[:, None]],
        axis=1).astype(np.float16)                       # [128, 42]
    b1b = np.broadcast_to(b1.astype(np.float16), (128, 128)).copy()
    b2b = np.broadcast_to(b2.astype(np.float32), (128, 40)).copy()
    dr1 = np.zeros((1, 132), np.float16)
    dr1[0, 128:132] = DUMMY_ASRC
    dr2 = np.zeros((1, 41), np.float16)
    dr2[0, 40] = DUMMY_ASRC

    nc = _get_program(nloc, nblk, npad, lay["K_prog"], lay["colofs"],
                      lay["sumk"])

    in_maps = []
    for c in range(N_CORES):
        xT_c = np.ascontiguousarray(
            xp[c * nloc:(c + 1) * nloc].T)               # [128, nloc] f16
        in_maps.append({
            "xT": xT_c,
            "offs": lay["offs"][c],
            "w1": W1.astype(np.float16),
            "att8": att8,
            "w2e": w2e,
            "b1b": b1b,
            "b2b": b2b,
            "dr1": dr1,
            "dr2": dr2,
        })

    res = run_bass_kernel_spmd(nc, in_maps, core_ids=list(range(N_CORES)))
    full = np.concatenate([res.results[c]["out"] for c in range(N_CORES)],
                          axis=0)                        # [npad, 40]
    return np.ascontiguousarray(full[nof[:n_real]]).astype(np.float32)
